# revision 26
# baseline (speedup 1.0000x reference)
"""Trainium2 Bass kernel for a 2-layer GAT (GATConv x2 + global mean pool + linear).

Strategy (8 NeuronCores, SPMD):
  - Nodes are dealt to cores by in-degree rank (rank r -> core r%8, slot r//8),
    so every core's slot s has ~equal degree => tight shared CSR capacity profile.
  - Edges live on the core that owns their DST. Per dst-group (128 slots) a
    padded CSR [128 dst x K columns] holds the in-edges.
  - Layer 1: the host ships x rows pre-expanded per edge token (pure data
    layout; bf16), and the kernel computes [h|logits] per CSR column with one
    TensorEngine matmul per 128-token column -- no gather at all. A 65th
    indicator feature row carries weight -1e30 into the src-logit so pad
    slots get exp(leakyrelu(...)) == 0 exactly.
  - Layer 2: per-node [h2|logits] are computed locally and AllGathered into a
    per-core DRAM table; per-edge rows are fetched with dma_gather (int16 idx
    => the 100352-row table is viewed in 4 chunks of <=32768 rows, each
    group's columns chunk-pure; pad slots point at -1e30 dummy rows).
  - Softmax runs per dst-partition row (max-subtraction is skipped: logits are
    O(1) so exp is safe in f32), aggregation is a broadcast-multiply + free-dim
    reduction on the Vector engine; layer outputs feed layer 2's table which is
    AllGathered again; mean-pool goes through a one-hot matmul on the Tensor
    engine and a final AllReduce.
"""
import sys, types, time

sys.path.insert(0, "/opt/trn_rl_repo")

import antenv  # noqa: E402
if not hasattr(antenv, "axon_hooks"):
    _m = types.ModuleType("antenv.axon_hooks")
    _m.get_axon_ntff_profile_hook = lambda: None
    sys.modules["antenv.axon_hooks"] = _m
    antenv.axon_hooks = _m

import numpy as np
import ml_dtypes
import concourse.bass as bass
import concourse.bacc as bacc
import concourse.tile as tile
import concourse.mybir as mybir
from concourse.masks import make_identity

# ---------------- problem constants (hardcoded; kernel.py must be self-contained)
N = 100000
IN = 64
HID = 16
HEADS = 4
G = 512
SLOPE = 0.2
NCORES = 8
SLOTS = 12544            # 98 groups x 128 (12500 real + 44 dummy slots per core)
NGRP = SLOTS // 128
NODESP = NCORES * SLOTS  # 100352 table rows
CHUNK_LIM = np.array([32768, 65536, 98304, NODESP])   # chunk of pid = searchsorted
VIEW_BASE = [0, 32768, 65536, 67584]                  # table view base per chunk
DUMMY_LOCAL = [12500, 4820, 9684, 32724]              # a dummy-slot row inside each view
COLS_PER_CALL = 7        # 896 tokens per dma_gather (57 descs; 2 in flight <= 128)
F32 = mybir.dt.float32
BF16 = mybir.dt.bfloat16
I16 = mybir.dt.int16


ACT = mybir.ActivationFunctionType
WB = 22528
WS = 32768
VIEW_BASE2 = [0, 22528, 45056, 67584]
DUMMY_LOCAL2 = [12500, 15060, 17620, 20180]   # a dummy-slot row inside each view
POOL = 32


# ---------------------------------------------------------------- host side
def _assign_nodes(src, dst):
    """Balanced node->(core, slot): pool-greedy on soft window counts."""
    deg = np.bincount(dst, minlength=N)
    order = np.argsort(-deg, kind="stable")
    eorder = np.argsort(src, kind="stable")
    es, ed = src[eorder], dst[eorder]
    out_start = np.searchsorted(es, np.arange(N + 1))

    npool = N // POOL
    pool_nodes = order[:npool * POOL].reshape(npool, POOL)
    rank = np.empty(N, np.int64)
    rank[order] = np.arange(N)
    core_arr = (rank % 8).astype(np.int64)
    slot_arr = (rank // 8).astype(np.int64)

    p_all = core_arr * SLOTS + slot_arr
    lo_all = np.maximum(0, (p_all - WS) // WB + 1)
    hi_all = np.minimum(3, p_all // WB)
    softW = np.zeros((N, 4), np.float32)
    softW[np.arange(N), lo_all] += np.where(lo_all == hi_all, 1.0, 0.5)
    softW[np.arange(N), hi_all] += np.where(lo_all == hi_all, 0.0, 0.5)
    inS = np.zeros((N, 4), np.float32)
    np.add.at(inS, ed, softW[es])

    pool_edges = []
    for b in range(npool):
        segs = [np.arange(out_start[n], out_start[n + 1]) for n in pool_nodes[b]]
        lens = np.array([len(s) for s in segs])
        idxs = np.concatenate(segs)
        owner = np.repeat(np.arange(POOL), lens)
        pool_edges.append((idxs, owner))

    s0 = (np.arange(npool) * POOL) // 8
    cells_core = np.repeat(np.arange(8), 4)
    for sweep in range(2):
        for b in range(npool):
            idxs, owner = pool_edges[b]
            nodes = pool_nodes[b]
            dsts = ed[idxs]
            np.subtract.at(inS, dsts, softW[nodes][owner])
            slots = s0[b] + np.arange(4)
            cells_pid = (np.repeat(np.arange(8), 4) * SLOTS + np.tile(slots, 8))
            lo = np.maximum(0, (cells_pid - WS) // WB + 1)
            hi = np.minimum(3, cells_pid // WB)
            cw = np.zeros((32, 4), np.float32)
            cw[np.arange(32), lo] += np.where(lo == hi, 1.0, 0.5)
            cw[np.arange(32), hi] += np.where(lo == hi, 0.0, 0.5)
            A = inS[dsts]
            Asum = np.zeros((POOL, 4), np.float32)
            for w in range(4):
                Asum[:, w] = np.bincount(owner, A[:, w], minlength=POOL)
            odeg = np.bincount(owner, minlength=POOL).astype(np.float32)
            cost = 2.0 * (Asum @ cw.T) + odeg[:, None] * (cw * cw).sum(1)[None, :]
            asg = np.full(POOL, -1, np.int32)
            used = np.zeros(32, bool)
            taken = 0
            for f in np.argsort(cost, axis=None, kind="stable"):
                i, cell = divmod(int(f), 32)
                if asg[i] >= 0 or used[cell]:
                    continue
                asg[i] = cell
                used[cell] = True
                taken += 1
                if taken == POOL:
                    break
            core_arr[nodes] = cells_core[asg]
            slot_arr[nodes] = slots[asg % 4]
            softW[nodes] = cw[asg]
            np.add.at(inS, dsts, softW[nodes][owner])
    return core_arr, slot_arr


def _route_chunks(pid, src, dst):
    """Per-edge chunk choice (overlapping windows), minimizing per-dst max."""
    deg = np.bincount(dst, minlength=N)
    spid = pid[src]
    cmin = np.maximum(0, (spid - WS) // WB + 1).astype(np.int8)
    cmax = np.minimum(3, spid // WB).astype(np.int8)
    flex = cmin < cmax
    fixed = np.zeros((N, 4), np.int32)
    np.add.at(fixed, (dst[~flex], cmin[~flex]), 1)
    flexc = np.zeros((N, 3), np.int32)
    np.add.at(flexc, (dst[flex], cmin[flex]), 1)

    def probe(T):
        ok = np.ones(N, bool)
        x = np.zeros((N, 3), np.int32)
        carry = np.zeros(N, np.int32)
        for w in range(4):
            load = fixed[:, w] + carry
            room = T - load
            ok &= room >= 0
            if w < 3:
                x[:, w] = np.minimum(flexc[:, w], np.maximum(room, 0))
                carry = flexc[:, w] - x[:, w]
        return ok, x

    x_fin = np.zeros((N, 3), np.int32)
    Tc = np.ceil(deg / 4).astype(np.int32)
    rem = np.ones(N, bool)
    for bump in range(16):
        ok, x = probe(Tc + bump)
        newly = rem & ok
        x_fin[newly] = x[newly]
        rem &= ~ok
        if not rem.any():
            break
    assert not rem.any()

    # materialize per-edge chunk: for each dst, flex edges of pair w take
    # x_fin[d, w] into w (the rest into w+1). Assign per (dst, pair) by rank.
    schunk = cmin.copy()
    fi = np.where(flex)[0]
    key = dst[fi] * 3 + cmin[fi]
    korder = np.argsort(key, kind="stable")
    ks = key[korder]
    kstart = np.searchsorted(ks, np.arange(N * 3 + 1))
    j = np.arange(len(ks)) - kstart[ks]          # rank within (dst, pair)
    quota = x_fin[dst[fi][korder], cmin[fi][korder]]
    up = j >= quota
    schunk_f = cmin[fi][korder].astype(np.int8)
    schunk_f[up] += 1
    schunk[fi[korder]] = schunk_f

    # ---- band-aware refinement: shave group-max chunk counts (token count
    # drives gather DMA, the dominant kernel cost). Move one flex edge per
    # driver (dst, chunk) cell per iteration to a chunk with band slack.
    band_d = ((pid % SLOTS) // 128).astype(np.int64)     # band of each node
    NB = SLOTS // 128
    alt = np.where(schunk[fi] == cmin[fi], cmax[fi], cmin[fi]).astype(np.int8)
    for _it in range(40):
        cnt_d = np.zeros((N, 4), np.int32)
        np.add.at(cnt_d, (dst, schunk), 1)
        Kgc_cur = np.zeros((NB, 4), np.int32)
        np.maximum.at(Kgc_cur, (band_d[:, None].repeat(4, 1),
                                np.arange(4)[None, :].repeat(N, 0)), cnt_d)
        ed_f = dst[fi]
        c_f = schunk[fi]
        bd_f = band_d[ed_f]
        driver = cnt_d[ed_f, c_f] == Kgc_cur[bd_f, c_f]
        room = cnt_d[ed_f, alt] + 1 < Kgc_cur[bd_f, alt]
        cand = driver & room & (alt != c_f)
        ci = np.where(cand)[0]
        if len(ci) == 0:
            break
        # capacity-limited moves: per source cell (dst, c) move at most
        # cnt - (target-1) edges; per dest cell (dst, alt) accept at most
        # Kgc[alt] - 1 - cnt edges.
        skey = ed_f[ci] * 4 + c_f[ci]
        so = np.argsort(skey, kind="stable")
        ss = skey[so]
        sstart = np.searchsorted(ss, np.arange(N * 4 + 1))
        srank = np.arange(len(ss)) - sstart[ss]
        src_cap = (cnt_d[ed_f[ci[so]], c_f[ci[so]]]
                   - (Kgc_cur[bd_f[ci[so]], c_f[ci[so]]] - 1))
        keep1 = srank < np.maximum(src_cap, 1)
        ci2 = ci[so][keep1]
        dkey = ed_f[ci2] * 4 + alt[ci2]
        do_ = np.argsort(dkey, kind="stable")
        ds = dkey[do_]
        dstart = np.searchsorted(ds, np.arange(N * 4 + 1))
        drank = np.arange(len(ds)) - dstart[ds]
        dst_cap = (Kgc_cur[bd_f[ci2[do_]], alt[ci2[do_]]] - 1
                   - cnt_d[ed_f[ci2[do_]], alt[ci2[do_]]])
        keep2 = drank < dst_cap
        mv = ci2[do_][keep2]
        if len(mv) == 0:
            break
        schunk[fi[mv]] = alt[mv]
        old = c_f[mv].copy()
        alt[mv] = old
    return schunk


def _host_prep2(x, edge_index, batch):
    # self-loops are handled analytically on-device (src==dst is local), so
    # they are excluded from the CSR: saves ~1 gather column per (band,chunk)
    # plus the direct tokens.
    src = np.ascontiguousarray(edge_index[0])
    dst = np.ascontiguousarray(edge_index[1])
    core_of, slot_of = _assign_nodes(src, dst)
    pid = core_of * SLOTS + slot_of
    schunk = _route_chunks(pid, src, dst)

    spid = pid[src]
    slocal = spid - np.array(VIEW_BASE2, np.int64)[schunk]
    assert (slocal >= 0).all() and (slocal < WS).all()
    dcore = core_of[dst]
    dslot = slot_of[dst]

    cnt = np.zeros((NCORES, SLOTS, 4), np.int32)
    np.add.at(cnt, (dcore, dslot, schunk), 1)
    gcnt = cnt.reshape(NCORES, NGRP, 128, 4)
    Kgc = gcnt.max(axis=(0, 2))
    Kgc[Kgc.sum(axis=1) == 0, 0] = 1     # keep tiles non-empty
    Kg = Kgc.sum(axis=1)
    coloff = np.zeros((NGRP, 4), np.int64)
    coloff[:, 1:] = np.cumsum(Kgc, axis=1)[:, :-1]
    ntok = int(Kg.sum()) * 128

    eorder = np.lexsort((spid, schunk, dslot, dcore))
    es, ed, ec, el = (a[eorder] for a in (spid, dslot, dcore, slocal))
    ech = schunk[eorder]
    key = (ed * 4 + ech) + ec * (SLOTS * 4)
    uniq, first = np.unique(key, return_index=True)
    rl = np.diff(np.append(first, len(key)))
    jrank = np.arange(len(key)) - np.repeat(first, rl)
    # quantile column placement: edge i of d_c sorted-by-spid edges goes to
    # column floor((i+0.5)*Kgc/d_c) so every column's 128 gathers hit a
    # narrow pid band (HBM row locality). Pads point at the (hot) dummy row.
    rl_e = np.repeat(rl, rl)
    Kc_e = Kgc[(ed // 128), ech]
    j = ((jrank * 2 + 1) * Kc_e // (rl_e * 2)).astype(np.int64)
    assert (j < Kc_e).all() and (j >= 0).all()

    idx16 = []
    gbase = np.concatenate([[0], np.cumsum(Kg)])
    for c in range(NCORES):
        m = ec == c
        gg = ed[m] // 128
        pp_ = ed[m] % 128
        col = gbase[gg] + coloff[gg, ech[m]] + j[m]
        colchunk = np.empty(int(Kg.sum()), np.int8)
        for g in range(NGRP):
            for ch in range(4):
                colchunk[gbase[g] + coloff[g, ch]:
                         gbase[g] + coloff[g, ch] + Kgc[g, ch]] = ch
        dl = np.array(DUMMY_LOCAL2, np.int16)[colchunk]
        val = np.repeat(dl, 128).astype(np.int16)
        val[col * 128 + pp_] = el[m].astype(np.int16)
        blk = val.reshape(-1, 16).T.astype(np.int16)
        idx16.append(np.ascontiguousarray(np.tile(blk, (8, 1))))

    xT = np.zeros((NCORES, IN, SLOTS), np.float32)
    for c in range(NCORES):
        nodes = np.where(core_of == c)[0]
        xT[c][:, slot_of[nodes]] = x[nodes].T

    poolind = np.zeros((NCORES, SLOTS, G), np.float32)
    for c in range(NCORES):
        nodes = np.where(core_of == c)[0]
        poolind[c, slot_of[nodes], batch[nodes]] = 1.0
    poolind = poolind.reshape(NCORES, NGRP, 128, G).astype(ml_dtypes.bfloat16)
    counts = np.bincount(batch, minlength=G).astype(np.float32)
    invcnt = np.tile((1.0 / np.maximum(counts, 1.0))[None, :], (HID, 1)).astype(np.float32)

    # layer-1 shipped expansion (no chunks)
    cnt1 = cnt.sum(axis=2)
    Kg1 = cnt1.reshape(NCORES, NGRP, 128).max(axis=(0, 2))
    Kg1 = np.maximum(Kg1, 1)
    gbase1 = np.concatenate([[0], np.cumsum(Kg1)])
    T1TOK = int(Kg1.sum()) * 128
    eorder1 = np.lexsort((dslot, dcore))
    ed1, ec1 = dslot[eorder1], dcore[eorder1]
    key1 = ec1 * SLOTS + ed1
    _, first1 = np.unique(key1, return_index=True)
    j1 = np.arange(len(key1)) - np.repeat(first1, np.diff(np.append(first1, len(key1))))
    xsrc = []
    for c in range(NCORES):
        m = ec1 == c
        gg = ed1[m] // 128
        pp_ = ed1[m] % 128
        tok = (gbase1[gg] + j1[m]) * 128 + pp_
        arr = np.zeros((T1TOK, 65), np.float32)
        arr[:, 64] = 1.0
        arr[tok, 0:64] = x[src[eorder1][m]]
        arr[tok, 64] = 0.0
        xsrc.append(np.ascontiguousarray(arr.T.astype(ml_dtypes.bfloat16)))

    meta = dict(Kgc=Kgc, Kg=Kg, coloff=coloff, gbase=gbase, ntok=ntok,
                Kg1=Kg1, gbase1=gbase1, T1TOK=T1TOK)
    return meta, idx16, xT, poolind, invcnt, xsrc


# ---------------------------------------------------------------- device build
def _build(meta, repeat=1, cfg=None):
    cfg = cfg or {}
    nqueues = cfg.get("nqueues", 3)
    Kgc, Kg, gbase, coloff = meta["Kgc"], meta["Kg"], meta["gbase"], meta["coloff"]
    NIDX16 = int(Kg.sum()) * 8
    W1E = 72
    W2E = 18
    TB2W = 128

    nc = bacc.Bacc(None, target_bir_lowering=False, num_swdge_queues=nqueues)
    xT = nc.declare_dram_parameter("xT", [IN, SLOTS], F32, isOutput=False)
    wext = nc.declare_dram_parameter("wext", [IN, W1E], F32, isOutput=False)
    w2ext = nc.declare_dram_parameter("w2ext", [IN, W2E], F32, isOutput=False)
    b1t = nc.declare_dram_parameter("b1t", [128, 64], F32, isOutput=False)
    b2t = nc.declare_dram_parameter("b2t", [128, HID], F32, isOutput=False)
    idx = nc.declare_dram_parameter("idx", [128, NIDX16], I16, isOutput=False)
    poolind = nc.declare_dram_parameter("poolind", [NGRP, 128, G], BF16, isOutput=False)
    invcnt = nc.declare_dram_parameter("invcnt", [HID, G], F32, isOutput=False)
    wc = nc.declare_dram_parameter("wc", [HID, 1], F32, isOutput=False)
    bc = nc.declare_dram_parameter("bc", [1, 1], F32, isOutput=False)
    xsrc = nc.declare_dram_parameter("xsrc", [65, int(meta["T1TOK"])], BF16, isOutput=False)
    wextb = nc.declare_dram_parameter("wextb", [65, 72], BF16, isOutput=False)
    negf = nc.declare_dram_parameter("negf", [44, 1], BF16, isOutput=False)
    out = nc.declare_dram_parameter("out", [1, G], F32, isOutput=True)

    table2 = nc.dram_tensor("table2", [NODESP, TB2W], BF16, addr_space="Shared")
    ar_out = nc.dram_tensor("ar_out", [HID, G], F32, addr_space="Shared")

    cc_ins = {}

    with tile.TileContext(nc) as tc:
        with (
            tc.tile_pool(name="persist", bufs=1) as pp,
            tc.tile_pool(name="work", bufs=2) as wp,
            tc.tile_pool(name="psum", bufs=2, space="PSUM") as psp,
            tc.tile_pool(name="psum1", bufs=1, space="PSUM") as psp1,
            tc.tile_pool(name="dram", bufs=1, space="DRAM") as dp,
        ):
            wext_sb = pp.tile([IN, W1E], F32)
            nc.sync.dma_start(wext_sb[:], wext[:, :])
            wextb_sb = pp.tile([65, 72], BF16)
            nc.sync.dma_start(wextb_sb[:], wextb[:, :])
            w2ext_sb = pp.tile([IN, W2E], F32)
            nc.sync.dma_start(w2ext_sb[:], w2ext[:, :])
            b1_sb = pp.tile([128, 64], F32)
            nc.sync.dma_start(b1_sb[:], b1t[:, :])
            b2_sb = pp.tile([128, HID], F32)
            nc.sync.dma_start(b2_sb[:], b2t[:, :])
            wc_sb = pp.tile([HID, 1], F32)
            nc.sync.dma_start(wc_sb[:], wc[:, :])
            bc_sb = pp.tile([1, 1], F32)
            nc.sync.dma_start(bc_sb[:], bc[:, :])
            invc_sb = pp.tile([HID, G], F32)
            nc.sync.dma_start(invc_sb[:], invcnt[:, :])
            ident = pp.tile([128, 128], F32)
            make_identity(nc, ident[:])
            alpha02 = pp.tile([128, 1], F32)
            nc.vector.memset(alpha02[:], SLOPE)

            # double-banked per-iteration tiles: lets iteration i+1's phase
            # A/B/C compute overlap iteration i's phase-D gather DMA
            ald_par = [pp.tile([128, NGRP, HEADS], F32, name=f"ald{i}") for i in range(2)]
            als_par = [pp.tile([128, NGRP, HEADS], F32, name=f"als{i}") for i in range(2)]
            h1s_par = [pp.tile([128, NGRP, 64], BF16, name=f"h1s{i}") for i in range(2)]
            w1s_par = [pp.tile([128, NGRP, HEADS], F32, name=f"w1s{i}") for i in range(2)]
            h2a_par = [pp.tile([128, NGRP, 17], BF16, name=f"h2a{i}") for i in range(2)]
            ls2_par = [pp.tile([128, NGRP], F32, name=f"ls2{i}") for i in range(2)]
            w2s_par = [pp.tile([128, NGRP, 1], F32, name=f"w2s{i}") for i in range(2)]
            z1_par = [pp.tile([128, NGRP, 64], F32, name=f"z1a{i}") for i in range(2)]
            ald2_par = [pp.tile([128, NGRP], F32, name=f"ald2{i}") for i in range(2)]
            z2_all = pp.tile([128, NGRP, HID], F32)
            t2in = dp.tile([SLOTS, TB2W], BF16)
            ar_in = dp.tile([HID, G], F32)

            for _rep in range(repeat):
              bk = _rep % 2
              ald2_sb = ald2_par[bk]
              ald_sb = ald_par[bk]
              als_sb = als_par[bk]
              h1self = h1s_par[bk]
              w1s_sb = w1s_par[bk]
              h2all = h2a_par[bk]
              ls2_sb = ls2_par[bk]
              w2s_sb = w2s_par[bk]
              z1_all = z1_par[bk]
              # ---------------- phase A
              for g in range(NGRP):
                xg = wp.tile([IN, 128], F32, tag="xg")
                nc.sync.dma_start(xg[:], xT[:, g * 128:(g + 1) * 128])
                ps = psp.tile([128, W1E], F32, tag="psA")
                nc.tensor.matmul(ps[:], xg[:], wext_sb[:], start=True, stop=True)
                nc.vector.tensor_copy(ald_sb[:, g, :], ps[:, 68:72])
                nc.vector.tensor_copy(als_sb[:, g, :], ps[:, 64:68])
                nc.scalar.activation(h1self[:, g, :], ps[:, 0:64], ACT.Copy)
              # batched self-loop weights for layer 1: w = exp(prelu(als+ald))
              e1s = wp.tile([128, NGRP * HEADS], F32, tag="e1s")
              nc.vector.tensor_add(
                  e1s[:].rearrange("p (g h) -> p g h", h=HEADS),
                  als_sb[:], ald_sb[:])
              nc.scalar.activation(e1s[:], e1s[:], ACT.Prelu, alpha=alpha02[:])
              nc.scalar.activation(
                  w1s_sb[:].rearrange("p g h -> p (g h)"), e1s[:], ACT.Exp)

              def edge_layer(table, TBW, nfeat, nheads, ald_t, bias_sb, zout, layer,
                             src_mm=False, selfw=None, selfh=None):
                alcol = nfeat if src_mm else 16
                L = layer
                Kmax = int(meta["Kg1"].max()) if src_mm else int(Kg.max())
                Kgl = meta["Kg1"] if src_mm else Kg
                gb1 = meta["gbase1"]
                for g in range(NGRP):
                    K = int(Kgl[g])
                    gt = wp.tile([128, Kmax * TBW], BF16, tag=f"gt{L}")
                    gtv = gt[:, 0:K * TBW].rearrange("p (k e) -> p k e", e=TBW)
                    if src_mm:
                        CB = 21
                        j0 = 0
                        while j0 < K:
                            j1 = min(j0 + CB, K)
                            xs = wp.tile([65, CB * 128], BF16, tag="xs")
                            nc.sync.dma_start(
                                xs[:, 0:(j1 - j0) * 128],
                                xsrc[:, (gb1[g] + j0) * 128:(gb1[g] + j1) * 128])
                            for j4 in range(j0, j1, 7):
                                j4e = min(j4 + 7, j1)
                                nj = j4e - j4
                                psb = psp.tile([128, 7 * W1E], F32, tag="psA")
                                for j in range(j4, j4e):
                                    nc.tensor.matmul(
                                        psb[:, (j - j4) * W1E:(j - j4 + 1) * W1E],
                                        xs[:, (j - j0) * 128:(j - j0 + 1) * 128],
                                        wextb_sb[:], start=True, stop=True)
                                nc.scalar.activation(
                                    gtv[:, j4:j4e, :],
                                    psb[:, 0:nj * W1E].rearrange(
                                        "p (j e) -> p j e", e=W1E)[:, :, 0:68],
                                    ACT.Copy)
                            j0 = j1
                    else:
                      idx_t = wp.tile([128, 8 * int(Kg.max())], I16, tag="idx")
                      nc.sync.dma_start(idx_t[:, 0:8 * K],
                                        idx[:, 8 * gbase[g]: 8 * (gbase[g] + K)])
                      for c in range(4):
                        kc = int(Kgc[g, c])
                        off = int(coloff[g, c])
                        base = VIEW_BASE2[c]
                        a = 0
                        while a < kc:
                            b = min(a + COLS_PER_CALL, kc)
                            T = (b - a) * 128
                            qn = cc_ins.setdefault("qctr", [0])
                            ins_g = nc.gpsimd.dma_gather(
                                gtv[:, off + a: off + b, :],
                                table[base: base + WS, :],
                                idx_t[:, 8 * (off + a): 8 * (off + b)],
                                T, T, TBW, queue_num=qn[0] % nqueues)
                            qn[0] += 1
                            cc_ins.setdefault(f"gathers{layer}", []).append(ins_g)
                            a = b

                    # softmax weights: Prelu(bias=ald)+Exp(accum) on scalar
                    e_t = wp.tile([128, nheads * Kmax], F32, tag=f"e{L}")
                    w_t = wp.tile([128, nheads * Kmax], BF16, tag=f"w{L}")
                    den = wp.tile([128, nheads], F32, tag=f"den{L}")
                    if nheads > 1:
                        # batch heads: DVE pre-add of ald, one Prelu + one Exp,
                        # denominators via DVE reduce (cuts ACT op count 4x)
                        ev3 = e_t[:, 0:nheads * K].rearrange(
                            "p (h k) -> p h k", k=K)
                        nc.vector.tensor_tensor(
                            out=ev3,
                            in0=gtv[:, :, alcol:alcol + nheads].rearrange(
                                "p k h -> p h k"),
                            in1=ald_t[:, g, :].rearrange(
                                "p (h o) -> p h o", o=1).to_broadcast(
                                [128, nheads, K]),
                            op=mybir.AluOpType.add)
                        nc.scalar.activation(e_t[:, 0:nheads * K],
                                             e_t[:, 0:nheads * K],
                                             ACT.Prelu, alpha=alpha02[:])
                        nc.scalar.activation(w_t[:, 0:nheads * K],
                                             e_t[:, 0:nheads * K], ACT.Exp)
                        nc.vector.reduce_sum(
                            den[:],
                            w_t[:, 0:nheads * K].rearrange(
                                "p (h k) -> p h k", k=K),
                            axis=mybir.AxisListType.X)
                    else:
                        for h in range(nheads):
                            ev = e_t[:, h * K:(h + 1) * K]
                            bias_ap = ald_t[:, g:g + 1]
                            nc.scalar.activation(ev, gtv[:, :, alcol + h],
                                                 ACT.Prelu, bias=bias_ap,
                                                 alpha=alpha02[:])
                            nc.scalar.activation(w_t[:, h * K:(h + 1) * K], ev,
                                                 ACT.Exp,
                                                 accum_out=den[:, h:h + 1])
                    # self-loop contribution to the softmax denominator
                    nc.vector.tensor_add(den[:], den[:], selfw[:, g, :])
                    dinv = wp.tile([128, nheads], F32, tag=f"dinv{L}")
                    nc.vector.reciprocal(dinv[:], den[:])

                    CH = nfeat // nheads
                    m_t = wp.tile([128, Kmax * nfeat], BF16, tag=f"m{L}")
                    mv = m_t[:, 0:K * nfeat].rearrange("p (k h c) -> p k h c",
                                                       h=nheads, c=CH)
                    wv = w_t[:, 0:nheads * K].rearrange("p (h k) -> p h k", k=K)
                    for h in range(nheads):
                        nc.vector.tensor_tensor(
                            out=mv[:, :, h, :],
                            in0=gtv[:, :, h * CH:(h + 1) * CH],
                            in1=wv[:, h:h + 1, :].rearrange(
                                "p o k -> p k o").to_broadcast([128, K, CH]),
                            op=mybir.AluOpType.mult)
                    msum = wp.tile([128, nfeat], F32, tag=f"ms{L}")
                    nc.vector.reduce_sum(
                        msum[:].rearrange("p (h c) -> p h c", c=CH),
                        mv.rearrange("p k h c -> p h c k"),
                        axis=mybir.AxisListType.X)
                    # self-loop contribution to the numerator
                    sm = wp.tile([128, nfeat], F32, tag=f"sm{L}")
                    nc.vector.tensor_tensor(
                        out=sm[:].rearrange("p (h c) -> p h c", c=CH),
                        in0=selfh[:, g, 0:nfeat].rearrange("p (h c) -> p h c", c=CH),
                        in1=selfw[:, g, :].rearrange("p (h o) -> p h o", o=1)
                        .to_broadcast([128, nheads, CH]),
                        op=mybir.AluOpType.mult)
                    nc.vector.tensor_add(msum[:], msum[:], sm[:])
                    y = wp.tile([128, nfeat], F32, tag=f"y{L}")
                    nc.vector.tensor_tensor(
                        out=y[:].rearrange("p (h c) -> p h c", c=CH),
                        in0=msum[:].rearrange("p (h c) -> p h c", c=CH),
                        in1=dinv[:].rearrange("p (h o) -> p h o", o=1).to_broadcast(
                            [128, nheads, CH]),
                        op=mybir.AluOpType.mult)
                    nc.vector.tensor_add(y[:], y[:], bias_sb[:])
                    tneg = wp.tile([128, nfeat], F32, tag=f"tn{L}")
                    nc.vector.tensor_scalar_min(tneg[:], y[:], 0.0)
                    ex = wp.tile([128, nfeat], F32, tag=f"ex{L}")
                    nc.scalar.activation(ex[:], tneg[:], ACT.Exp)
                    nc.vector.tensor_scalar_add(ex[:], ex[:], -1.0)
                    nc.vector.tensor_scalar_max(y[:], y[:], 0.0)
                    nc.vector.tensor_add(zout[:, g, :], y[:], ex[:])

              # ---------------- phase B
              edge_layer(None, 68, 64, HEADS, ald_sb, b1_sb, z1_all, 1, src_mm=True,
                         selfw=w1s_sb, selfh=h1self)

              # ---------------- phase C
              for g in range(NGRP):
                pst = psp.tile([64, 128], F32, tag="psT")
                nc.tensor.transpose(pst[:], z1_all[:, g, :], ident[:])
                z1T = wp.tile([64, 128], F32, tag="z1T")
                nc.vector.tensor_copy(z1T[:], pst[:])
                ps2 = psp.tile([128, W2E], F32, tag="ps2")
                nc.tensor.matmul(ps2[:], z1T[:], w2ext_sb[:], start=True, stop=True)
                nc.scalar.activation(h2all[:, g, :], ps2[:, 0:17], ACT.Copy)
                nc.vector.tensor_copy(ald2_sb[:, g:g + 1], ps2[:, 17:18])
                nc.vector.tensor_copy(ls2_sb[:, g:g + 1], ps2[:, 16:17])
                nc.sync.dma_start(t2in[g * 128:(g + 1) * 128, 0:17], h2all[:, g, :])
              nc.sync.dma_start(t2in[12500:12544, 16:17], negf[:, :])
              # batched self-loop weights for layer 2
              e2s = wp.tile([128, NGRP], F32, tag="e2s")
              nc.vector.tensor_add(e2s[:], ls2_sb[:], ald2_sb[:])
              nc.scalar.activation(e2s[:], e2s[:], ACT.Prelu, alpha=alpha02[:])
              nc.scalar.activation(
                  w2s_sb[:].rearrange("p g h -> p (g h)"), e2s[:], ACT.Exp)
              cc2 = nc.gpsimd.collective_compute(
                "AllGather", mybir.AluOpType.bypass,
                replica_groups=[list(range(NCORES))],
                ins=[t2in[:].opt()], outs=[table2[:, :].opt()])
              cc_ins["ag2"] = cc2.ins if hasattr(cc2, "ins") else cc2
              from concourse.bass import _add_dep_helper
              for pg in cc_ins.get("prev_g2", []):
                  _add_dep_helper(cc_ins["ag2"], pg.ins, sync=True)

              # ---------------- phase D
              edge_layer(table2, TB2W, HID, 1, ald2_sb, b2_sb, z2_all, 2,
                         selfw=w2s_sb, selfh=h2all)

              # ---------------- phase E
              pool_ps = psp1.tile([HID, G], F32)
              for g in range(NGRP):
                z2b = wp.tile([128, HID], BF16, tag="z2b")
                nc.vector.tensor_copy(z2b[:], z2_all[:, g, :])
                pind = wp.tile([128, G], BF16, tag="pind")
                nc.sync.dma_start(pind[:], poolind[g, :, :])
                nc.tensor.matmul(pool_ps[:], z2b[:], pind[:],
                                 start=(g == 0), stop=(g == NGRP - 1))
              pool_sb = pp.tile([HID, G], F32)
              nc.vector.tensor_copy(pool_sb[:], pool_ps[:])
              nc.sync.dma_start(ar_in[:], pool_sb[:])
              cc3 = nc.gpsimd.collective_compute(
                "AllReduce", mybir.AluOpType.add,
                replica_groups=[list(range(NCORES))],
                ins=[ar_in[:].opt()], outs=[ar_out[:, :].opt()])
              cc_ins["ar"] = cc3.ins if hasattr(cc3, "ins") else cc3

              pooled = pp.tile([HID, G], F32)
              dma_pool = nc.sync.dma_start(pooled[:], ar_out[:, :])
              nc.vector.tensor_mul(pooled[:], pooled[:], invc_sb[:])
              ps_out = psp1.tile([1, G], F32)
              nc.tensor.matmul(ps_out[:], wc_sb[:], pooled[:], start=True, stop=True)
              res = pp.tile([1, G], F32)
              nc.vector.tensor_tensor(out=res[:], in0=ps_out[:],
                                    in1=bc_sb[:].to_broadcast([1, G]),
                                    op=mybir.AluOpType.add)
              nc.sync.dma_start(out[:, :], res[:])

              from concourse.bass import _add_dep_helper
              for gg in cc_ins.get("gathers2", []):
                  _add_dep_helper(gg.ins, cc_ins["ag2"], sync=True)
              _add_dep_helper(dma_pool.ins, cc_ins["ar"], sync=True)
              cc_ins["prev_g2"] = cc_ins.get("gathers2", [])[-nqueues:]
              cc_ins.pop("gathers2", None)
              cc_ins.pop("ag2", None)
              cc_ins.pop("ar", None)

    nc.finalize()
    return nc


# ---------------------------------------------------------------- runner
def _make_spmd_fn(nc, n_cores=8):
    import jax
    from concourse.bass2jax import (_bass_exec_p, install_neuronx_cc_hook,
                                    partition_id_tensor)
    from jax.sharding import Mesh, PartitionSpec, NamedSharding
    from jax.experimental.shard_map import shard_map

    install_neuronx_cc_hook()
    partition_name = nc.partition_id_tensor.name if nc.partition_id_tensor else None
    in_names, out_names, out_avals, zero_outs = [], [], [], []
    for alloc in nc.m.functions[0].allocations:
        if not isinstance(alloc, mybir.MemoryLocationSet):
            continue
        name = alloc.memorylocations[0].name
        if alloc.kind == "ExternalInput":
            if name != partition_name:
                in_names.append(name)
        elif alloc.kind == "ExternalOutput":
            out_names.append(name)
            shape = tuple(alloc.tensor_shape)
            dtype = mybir.dt.np(alloc.dtype)
            out_avals.append(jax.core.ShapedArray(shape, dtype))
            zero_outs.append(np.zeros(shape, dtype))
    n_params = len(in_names)
    all_in = list(in_names) + list(out_names)
    if partition_name is not None:
        all_in.append(partition_name)

    def _body(*args):
        operands = list(args)
        if partition_name is not None:
            operands.append(partition_id_tensor())
        return tuple(_bass_exec_p.bind(
            *operands, out_avals=tuple(out_avals), in_names=tuple(all_in),
            out_names=tuple(out_names), lowering_input_output_aliases=(),
            sim_require_finite=False, sim_require_nnan=False, nc=nc))

    devices = jax.devices()[:n_cores]
    mesh = Mesh(np.asarray(devices), ("core",))
    specs = (PartitionSpec("core"),)
    sharded = jax.jit(
        shard_map(_body, mesh=mesh, in_specs=specs * (n_params + len(out_names)),
                  out_specs=specs * len(out_names), check_rep=False),
        keep_unused=True)

    def place(in_maps):
        sh = NamedSharding(mesh, PartitionSpec("core"))
        placed = [jax.device_put(
            np.concatenate([np.asarray(in_maps[c][nm]) for c in range(n_cores)], axis=0), sh)
            for nm in in_names]
        jax.block_until_ready(placed)
        return placed

    def fn(in_maps_or_placed):
        if isinstance(in_maps_or_placed, list) and in_maps_or_placed and                 isinstance(in_maps_or_placed[0], dict):
            concat = place(in_maps_or_placed)
        else:
            concat = in_maps_or_placed
        zeros = [np.zeros((n_cores * z.shape[0], *z.shape[1:]), z.dtype)
                 for z in zero_outs]
        outs = sharded(*concat, *zeros)
        jax.block_until_ready(outs)
        return [{nm: np.asarray(outs[i]).reshape(n_cores, *out_avals[i].shape)[c]
                 for i, nm in enumerate(out_names)} for c in range(n_cores)]

    fn.place = place
    return fn



def make_inputs(inputs):
    x = np.asarray(inputs["x"], np.float32)
    edge_index = np.asarray(inputs["edge_index"], np.int64)
    batch = np.asarray(inputs["batch"], np.int64)
    W1 = np.asarray(inputs["W1"], np.float32)
    a1_src = np.asarray(inputs["a1_src"], np.float32)
    a1_dst = np.asarray(inputs["a1_dst"], np.float32)
    W2 = np.asarray(inputs["W2"], np.float32)
    a2_src = np.asarray(inputs["a2_src"], np.float32)
    a2_dst = np.asarray(inputs["a2_dst"], np.float32)

    meta, idx16, xT, poolind, invcnt, xsrc = _host_prep2(x, edge_index, batch)

    wext = np.zeros((IN, 72), np.float32)
    wext[:, 0:64] = W1
    for h in range(HEADS):
        wext[:, 64 + h] = W1[:, h * HID:(h + 1) * HID] @ a1_src[h]
        wext[:, 68 + h] = W1[:, h * HID:(h + 1) * HID] @ a1_dst[h]
    w2ext = np.zeros((IN, 18), np.float32)
    w2ext[:, 0:16] = W2
    w2ext[:, 16] = W2 @ a2_src[0]
    w2ext[:, 17] = W2 @ a2_dst[0]
    wextb = np.zeros((65, 72), np.float32)
    wextb[0:64, :] = wext
    wextb[64, 64:68] = -1e30
    wextb = wextb.astype(ml_dtypes.bfloat16)
    b1t = np.tile(np.asarray(inputs["b1"], np.float32)[None, :], (128, 1))
    b2t = np.tile(np.asarray(inputs["b2"], np.float32)[None, :], (128, 1))

    in_maps = []
    for c in range(NCORES):
        in_maps.append(dict(
            xT=xT[c], wext=wext, w2ext=w2ext, b1t=b1t, b2t=b2t,
            idx=idx16[c], poolind=poolind[c], invcnt=invcnt,
            wc=np.asarray(inputs["Wc"], np.float32).reshape(HID, 1),
            bc=np.asarray(inputs["bc"], np.float32).reshape(1, 1),
            xsrc=xsrc[c], wextb=wextb,
            negf=np.full((44, 1), -1e30, ml_dtypes.bfloat16)))
    return meta, in_maps


def kernel(**inputs):
    meta, in_maps = make_inputs(inputs)
    nc = _build(meta)
    fn = _make_spmd_fn(nc)
    placed = fn.place(in_maps)
    prev = None
    outv = None
    for _ in range(3):
        res = fn(placed)
        cur = res[0]["out"].reshape(G, 1).astype(np.float32)
        if prev is not None and np.allclose(cur, prev, rtol=1e-5, atol=1e-8):
            outv = cur
            break
        prev = cur
    if outv is None:
        outv = prev
    kernel._last_fn = fn
    kernel._last_meta = meta
    kernel._last_in_maps = in_maps
    return outv



# revision 29
# speedup vs baseline: 1.2209x; 1.2209x over previous
"""Trainium2 Bass kernel for a 2-layer GAT (GATConv x2 + global mean pool + linear).

Strategy (8 NeuronCores, SPMD):
  - Nodes are dealt to cores by in-degree rank (rank r -> core r%8, slot r//8),
    so every core's slot s has ~equal degree => tight shared CSR capacity profile.
  - Edges live on the core that owns their DST. Per dst-group (128 slots) a
    padded CSR [128 dst x K columns] holds the in-edges.
  - Layer 1: the host ships x rows pre-expanded per edge token (pure data
    layout; bf16), and the kernel computes [h|logits] per CSR column with one
    TensorEngine matmul per 128-token column -- no gather at all. A 65th
    indicator feature row carries weight -1e30 into the src-logit so pad
    slots get exp(leakyrelu(...)) == 0 exactly.
  - Layer 2: per-node [h2|logits] are computed locally and AllGathered into a
    per-core DRAM table; per-edge rows are fetched with dma_gather (int16 idx
    => the 100352-row table is viewed in 4 chunks of <=32768 rows, each
    group's columns chunk-pure; pad slots point at -1e30 dummy rows).
  - Softmax runs per dst-partition row (max-subtraction is skipped: logits are
    O(1) so exp is safe in f32), aggregation is a broadcast-multiply + free-dim
    reduction on the Vector engine; layer outputs feed layer 2's table which is
    AllGathered again; mean-pool goes through a one-hot matmul on the Tensor
    engine and a final AllReduce.
"""
import sys, types, time

sys.path.insert(0, "/opt/trn_rl_repo")

import antenv  # noqa: E402
if not hasattr(antenv, "axon_hooks"):
    _m = types.ModuleType("antenv.axon_hooks")
    _m.get_axon_ntff_profile_hook = lambda: None
    sys.modules["antenv.axon_hooks"] = _m
    antenv.axon_hooks = _m

import numpy as np
import ml_dtypes
import concourse.bass as bass
import concourse.bacc as bacc
import concourse.tile as tile
import concourse.mybir as mybir
from concourse.masks import make_identity

# ---------------- problem constants (hardcoded; kernel.py must be self-contained)
N = 100000
IN = 64
HID = 16
HEADS = 4
G = 512
SLOPE = 0.2
NCORES = 8
SLOTS = 12544            # 98 groups x 128 (12500 real + 44 dummy slots per core)
NGRP = SLOTS // 128
NODESP = NCORES * SLOTS  # 100352 table rows
CHUNK_LIM = np.array([32768, 65536, 98304, NODESP])   # chunk of pid = searchsorted
VIEW_BASE = [0, 32768, 65536, 67584]                  # table view base per chunk
DUMMY_LOCAL = [12500, 4820, 9684, 32724]              # a dummy-slot row inside each view
COLS_PER_CALL = 7        # 896 tokens per dma_gather (57 descs; 2 in flight <= 128)
F32 = mybir.dt.float32
BF16 = mybir.dt.bfloat16
I16 = mybir.dt.int16


ACT = mybir.ActivationFunctionType
WB = 22528
WS = 32768
VIEW_BASE2 = [0, 22528, 45056, 67584]
DUMMY_LOCAL2 = [12500, 15060, 17620, 20180]   # a dummy-slot row inside each view
POOL = 32


# ---------------------------------------------------------------- host side
def _assign_nodes(src, dst):
    """Balanced node->(core, slot): pool-greedy on soft window counts."""
    deg = np.bincount(dst, minlength=N)
    order = np.argsort(-deg, kind="stable")
    eorder = np.argsort(src, kind="stable")
    es, ed = src[eorder], dst[eorder]
    out_start = np.searchsorted(es, np.arange(N + 1))

    npool = N // POOL
    pool_nodes = order[:npool * POOL].reshape(npool, POOL)
    rank = np.empty(N, np.int64)
    rank[order] = np.arange(N)
    core_arr = (rank % 8).astype(np.int64)
    slot_arr = (rank // 8).astype(np.int64)

    p_all = core_arr * SLOTS + slot_arr
    lo_all = np.maximum(0, (p_all - WS) // WB + 1)
    hi_all = np.minimum(3, p_all // WB)
    softW = np.zeros((N, 4), np.float32)
    softW[np.arange(N), lo_all] += np.where(lo_all == hi_all, 1.0, 0.5)
    softW[np.arange(N), hi_all] += np.where(lo_all == hi_all, 0.0, 0.5)
    inS = np.zeros((N, 4), np.float32)
    np.add.at(inS, ed, softW[es])

    pool_edges = []
    for b in range(npool):
        segs = [np.arange(out_start[n], out_start[n + 1]) for n in pool_nodes[b]]
        lens = np.array([len(s) for s in segs])
        idxs = np.concatenate(segs)
        owner = np.repeat(np.arange(POOL), lens)
        pool_edges.append((idxs, owner))

    s0 = (np.arange(npool) * POOL) // 8
    cells_core = np.repeat(np.arange(8), 4)
    for sweep in range(2):
        for b in range(npool):
            idxs, owner = pool_edges[b]
            nodes = pool_nodes[b]
            dsts = ed[idxs]
            np.subtract.at(inS, dsts, softW[nodes][owner])
            slots = s0[b] + np.arange(4)
            cells_pid = (np.repeat(np.arange(8), 4) * SLOTS + np.tile(slots, 8))
            lo = np.maximum(0, (cells_pid - WS) // WB + 1)
            hi = np.minimum(3, cells_pid // WB)
            cw = np.zeros((32, 4), np.float32)
            cw[np.arange(32), lo] += np.where(lo == hi, 1.0, 0.5)
            cw[np.arange(32), hi] += np.where(lo == hi, 0.0, 0.5)
            A = inS[dsts]
            Asum = np.zeros((POOL, 4), np.float32)
            for w in range(4):
                Asum[:, w] = np.bincount(owner, A[:, w], minlength=POOL)
            odeg = np.bincount(owner, minlength=POOL).astype(np.float32)
            cost = 2.0 * (Asum @ cw.T) + odeg[:, None] * (cw * cw).sum(1)[None, :]
            asg = np.full(POOL, -1, np.int32)
            used = np.zeros(32, bool)
            taken = 0
            for f in np.argsort(cost, axis=None, kind="stable"):
                i, cell = divmod(int(f), 32)
                if asg[i] >= 0 or used[cell]:
                    continue
                asg[i] = cell
                used[cell] = True
                taken += 1
                if taken == POOL:
                    break
            core_arr[nodes] = cells_core[asg]
            slot_arr[nodes] = slots[asg % 4]
            softW[nodes] = cw[asg]
            np.add.at(inS, dsts, softW[nodes][owner])
    return core_arr, slot_arr


def _route_chunks(pid, src, dst):
    """Per-edge chunk choice (overlapping windows), minimizing per-dst max."""
    deg = np.bincount(dst, minlength=N)
    spid = pid[src]
    cmin = np.maximum(0, (spid - WS) // WB + 1).astype(np.int8)
    cmax = np.minimum(3, spid // WB).astype(np.int8)
    flex = cmin < cmax
    fixed = np.zeros((N, 4), np.int32)
    np.add.at(fixed, (dst[~flex], cmin[~flex]), 1)
    flexc = np.zeros((N, 3), np.int32)
    np.add.at(flexc, (dst[flex], cmin[flex]), 1)

    def probe(T):
        ok = np.ones(N, bool)
        x = np.zeros((N, 3), np.int32)
        carry = np.zeros(N, np.int32)
        for w in range(4):
            load = fixed[:, w] + carry
            room = T - load
            ok &= room >= 0
            if w < 3:
                x[:, w] = np.minimum(flexc[:, w], np.maximum(room, 0))
                carry = flexc[:, w] - x[:, w]
        return ok, x

    x_fin = np.zeros((N, 3), np.int32)
    Tc = np.ceil(deg / 4).astype(np.int32)
    rem = np.ones(N, bool)
    for bump in range(16):
        ok, x = probe(Tc + bump)
        newly = rem & ok
        x_fin[newly] = x[newly]
        rem &= ~ok
        if not rem.any():
            break
    assert not rem.any()

    # materialize per-edge chunk: for each dst, flex edges of pair w take
    # x_fin[d, w] into w (the rest into w+1). Assign per (dst, pair) by rank.
    schunk = cmin.copy()
    fi = np.where(flex)[0]
    key = dst[fi] * 3 + cmin[fi]
    korder = np.argsort(key, kind="stable")
    ks = key[korder]
    kstart = np.searchsorted(ks, np.arange(N * 3 + 1))
    j = np.arange(len(ks)) - kstart[ks]          # rank within (dst, pair)
    quota = x_fin[dst[fi][korder], cmin[fi][korder]]
    up = j >= quota
    schunk_f = cmin[fi][korder].astype(np.int8)
    schunk_f[up] += 1
    schunk[fi[korder]] = schunk_f

    # ---- band-aware refinement: shave group-max chunk counts (token count
    # drives gather DMA, the dominant kernel cost). Move one flex edge per
    # driver (dst, chunk) cell per iteration to a chunk with band slack.
    band_d = ((pid % SLOTS) // 128).astype(np.int64)     # band of each node
    NB = SLOTS // 128
    alt = np.where(schunk[fi] == cmin[fi], cmax[fi], cmin[fi]).astype(np.int8)
    for _it in range(40):
        cnt_d = np.zeros((N, 4), np.int32)
        np.add.at(cnt_d, (dst, schunk), 1)
        Kgc_cur = np.zeros((NB, 4), np.int32)
        np.maximum.at(Kgc_cur, (band_d[:, None].repeat(4, 1),
                                np.arange(4)[None, :].repeat(N, 0)), cnt_d)
        ed_f = dst[fi]
        c_f = schunk[fi]
        bd_f = band_d[ed_f]
        driver = cnt_d[ed_f, c_f] == Kgc_cur[bd_f, c_f]
        room = cnt_d[ed_f, alt] + 1 < Kgc_cur[bd_f, alt]
        cand = driver & room & (alt != c_f)
        ci = np.where(cand)[0]
        if len(ci) == 0:
            break
        # capacity-limited moves: per source cell (dst, c) move at most
        # cnt - (target-1) edges; per dest cell (dst, alt) accept at most
        # Kgc[alt] - 1 - cnt edges.
        skey = ed_f[ci] * 4 + c_f[ci]
        so = np.argsort(skey, kind="stable")
        ss = skey[so]
        sstart = np.searchsorted(ss, np.arange(N * 4 + 1))
        srank = np.arange(len(ss)) - sstart[ss]
        src_cap = (cnt_d[ed_f[ci[so]], c_f[ci[so]]]
                   - (Kgc_cur[bd_f[ci[so]], c_f[ci[so]]] - 1))
        keep1 = srank < np.maximum(src_cap, 1)
        ci2 = ci[so][keep1]
        dkey = ed_f[ci2] * 4 + alt[ci2]
        do_ = np.argsort(dkey, kind="stable")
        ds = dkey[do_]
        dstart = np.searchsorted(ds, np.arange(N * 4 + 1))
        drank = np.arange(len(ds)) - dstart[ds]
        dst_cap = (Kgc_cur[bd_f[ci2[do_]], alt[ci2[do_]]] - 1
                   - cnt_d[ed_f[ci2[do_]], alt[ci2[do_]]])
        keep2 = drank < dst_cap
        mv = ci2[do_][keep2]
        if len(mv) == 0:
            break
        schunk[fi[mv]] = alt[mv]
        old = c_f[mv].copy()
        alt[mv] = old
    return schunk


def _host_prep2(x, edge_index, batch):
    # self-loops are handled analytically on-device (src==dst is local), so
    # they are excluded from the CSR: saves ~1 gather column per (band,chunk)
    # plus the direct tokens.
    src = np.ascontiguousarray(edge_index[0])
    dst = np.ascontiguousarray(edge_index[1])
    core_of, slot_of = _assign_nodes(src, dst)
    pid = core_of * SLOTS + slot_of
    schunk = _route_chunks(pid, src, dst)

    spid = pid[src]
    slocal = spid - np.array(VIEW_BASE2, np.int64)[schunk]
    assert (slocal >= 0).all() and (slocal < WS).all()
    dcore = core_of[dst]
    dslot = slot_of[dst]

    cnt = np.zeros((NCORES, SLOTS, 4), np.int32)
    np.add.at(cnt, (dcore, dslot, schunk), 1)
    gcnt = cnt.reshape(NCORES, NGRP, 128, 4)
    Kgc = gcnt.max(axis=(0, 2))
    Kgc[Kgc.sum(axis=1) == 0, 0] = 1     # keep tiles non-empty
    Kg = Kgc.sum(axis=1)
    coloff = np.zeros((NGRP, 4), np.int64)
    coloff[:, 1:] = np.cumsum(Kgc, axis=1)[:, :-1]
    ntok = int(Kg.sum()) * 128

    eorder = np.lexsort((spid, schunk, dslot, dcore))
    es, ed, ec, el = (a[eorder] for a in (spid, dslot, dcore, slocal))
    ech = schunk[eorder]
    key = (ed * 4 + ech) + ec * (SLOTS * 4)
    uniq, first = np.unique(key, return_index=True)
    rl = np.diff(np.append(first, len(key)))
    jrank = np.arange(len(key)) - np.repeat(first, rl)
    # quantile column placement: edge i of d_c sorted-by-spid edges goes to
    # column floor((i+0.5)*Kgc/d_c) so every column's 128 gathers hit a
    # narrow pid band (HBM row locality). Pads point at the (hot) dummy row.
    rl_e = np.repeat(rl, rl)
    Kc_e = Kgc[(ed // 128), ech]
    j = ((jrank * 2 + 1) * Kc_e // (rl_e * 2)).astype(np.int64)
    assert (j < Kc_e).all() and (j >= 0).all()

    idx16 = []
    gbase = np.concatenate([[0], np.cumsum(Kg)])
    for c in range(NCORES):
        m = ec == c
        gg = ed[m] // 128
        pp_ = ed[m] % 128
        col = gbase[gg] + coloff[gg, ech[m]] + j[m]
        colchunk = np.empty(int(Kg.sum()), np.int8)
        for g in range(NGRP):
            for ch in range(4):
                colchunk[gbase[g] + coloff[g, ch]:
                         gbase[g] + coloff[g, ch] + Kgc[g, ch]] = ch
        dl = np.array(DUMMY_LOCAL2, np.int16)[colchunk]
        val = np.repeat(dl, 128).astype(np.int16)
        val[col * 128 + pp_] = el[m].astype(np.int16)
        blk = val.reshape(-1, 16).T.astype(np.int16)
        idx16.append(np.ascontiguousarray(np.tile(blk, (8, 1))))

    xT = np.zeros((NCORES, IN, SLOTS), np.float32)
    for c in range(NCORES):
        nodes = np.where(core_of == c)[0]
        xT[c][:, slot_of[nodes]] = x[nodes].T

    poolind = np.zeros((NCORES, SLOTS, G), np.float32)
    for c in range(NCORES):
        nodes = np.where(core_of == c)[0]
        poolind[c, slot_of[nodes], batch[nodes]] = 1.0
    poolind = poolind.reshape(NCORES, NGRP, 128, G).astype(ml_dtypes.bfloat16)
    counts = np.bincount(batch, minlength=G).astype(np.float32)
    invcnt = np.tile((1.0 / np.maximum(counts, 1.0))[None, :], (HID, 1)).astype(np.float32)

    # layer-1 shipped expansion (no chunks)
    cnt1 = cnt.sum(axis=2)
    Kg1 = cnt1.reshape(NCORES, NGRP, 128).max(axis=(0, 2))
    Kg1 = np.maximum(Kg1, 1)
    gbase1 = np.concatenate([[0], np.cumsum(Kg1)])
    T1TOK = int(Kg1.sum()) * 128
    eorder1 = np.lexsort((dslot, dcore))
    ed1, ec1 = dslot[eorder1], dcore[eorder1]
    key1 = ec1 * SLOTS + ed1
    _, first1 = np.unique(key1, return_index=True)
    j1 = np.arange(len(key1)) - np.repeat(first1, np.diff(np.append(first1, len(key1))))
    xsrc = []
    for c in range(NCORES):
        m = ec1 == c
        gg = ed1[m] // 128
        pp_ = ed1[m] % 128
        tok = (gbase1[gg] + j1[m]) * 128 + pp_
        arr = np.zeros((T1TOK, 65), np.float32)
        arr[:, 64] = 1.0
        arr[tok, 0:64] = x[src[eorder1][m]]
        arr[tok, 64] = 0.0
        xsrc.append(np.ascontiguousarray(arr.T.astype(ml_dtypes.bfloat16)))

    meta = dict(Kgc=Kgc, Kg=Kg, coloff=coloff, gbase=gbase, ntok=ntok,
                Kg1=Kg1, gbase1=gbase1, T1TOK=T1TOK)
    return meta, idx16, xT, poolind, invcnt, xsrc


# ---------------------------------------------------------------- device build
def _build(meta, repeat=1, cfg=None):
    cfg = cfg or {}
    nqueues = cfg.get("nqueues", 3)
    Kgc, Kg, gbase, coloff = meta["Kgc"], meta["Kg"], meta["gbase"], meta["coloff"]
    NIDX16 = int(Kg.sum()) * 8
    W1E = 72
    W2E = 18
    TB2W = 128

    nc = bacc.Bacc(None, target_bir_lowering=False, num_swdge_queues=nqueues)
    xT = nc.declare_dram_parameter("xT", [IN, SLOTS], F32, isOutput=False)
    wext = nc.declare_dram_parameter("wext", [IN, W1E], F32, isOutput=False)
    w2ext = nc.declare_dram_parameter("w2ext", [IN, W2E], F32, isOutput=False)
    b1t = nc.declare_dram_parameter("b1t", [128, 64], F32, isOutput=False)
    b2t = nc.declare_dram_parameter("b2t", [128, HID], F32, isOutput=False)
    idx = nc.declare_dram_parameter("idx", [128, NIDX16], I16, isOutput=False)
    poolind = nc.declare_dram_parameter("poolind", [NGRP, 128, G], BF16, isOutput=False)
    invcnt = nc.declare_dram_parameter("invcnt", [HID, G], F32, isOutput=False)
    wc = nc.declare_dram_parameter("wc", [HID, 1], F32, isOutput=False)
    bc = nc.declare_dram_parameter("bc", [1, 1], F32, isOutput=False)
    xsrc = nc.declare_dram_parameter("xsrc", [65, int(meta["T1TOK"])], BF16, isOutput=False)
    wextb = nc.declare_dram_parameter("wextb", [65, 72], BF16, isOutput=False)
    negf = nc.declare_dram_parameter("negf", [44, 1], BF16, isOutput=False)
    out = nc.declare_dram_parameter("out", [1, G], F32, isOutput=True)

    table2 = nc.dram_tensor("table2", [NODESP, TB2W], BF16, addr_space="Shared")
    ar_out = nc.dram_tensor("ar_out", [HID, G], F32, addr_space="Shared")

    cc_ins = {}

    with tile.TileContext(nc) as tc:
        with (
            tc.tile_pool(name="persist", bufs=1) as pp,
            tc.tile_pool(name="work", bufs=2) as wp,
            tc.tile_pool(name="psum", bufs=2, space="PSUM") as psp,
            tc.tile_pool(name="psum1", bufs=1, space="PSUM") as psp1,
            tc.tile_pool(name="dram", bufs=1, space="DRAM") as dp,
        ):
            wext_sb = pp.tile([IN, W1E], F32)
            nc.sync.dma_start(wext_sb[:], wext[:, :])
            wextb_sb = pp.tile([65, 72], BF16)
            nc.sync.dma_start(wextb_sb[:], wextb[:, :])
            w2ext_sb = pp.tile([IN, W2E], F32)
            nc.sync.dma_start(w2ext_sb[:], w2ext[:, :])
            b1_sb = pp.tile([128, 64], F32)
            nc.sync.dma_start(b1_sb[:], b1t[:, :])
            b2_sb = pp.tile([128, HID], F32)
            nc.sync.dma_start(b2_sb[:], b2t[:, :])
            wc_sb = pp.tile([HID, 1], F32)
            nc.sync.dma_start(wc_sb[:], wc[:, :])
            bc_sb = pp.tile([1, 1], F32)
            nc.sync.dma_start(bc_sb[:], bc[:, :])
            invc_sb = pp.tile([HID, G], F32)
            nc.sync.dma_start(invc_sb[:], invcnt[:, :])
            ident = pp.tile([128, 128], F32)
            make_identity(nc, ident[:])
            alpha02 = pp.tile([128, 1], F32)
            nc.vector.memset(alpha02[:], SLOPE)

            # double-banked per-iteration tiles: lets iteration i+1's phase
            # A/B/C compute overlap iteration i's phase-D gather DMA
            ald_par = [pp.tile([128, NGRP, HEADS], F32, name=f"ald{i}") for i in range(2)]
            als_par = [pp.tile([128, NGRP, HEADS], F32, name=f"als{i}") for i in range(2)]
            h1s_par = [pp.tile([128, NGRP, 64], BF16, name=f"h1s{i}") for i in range(2)]
            w1s_par = [pp.tile([128, NGRP, HEADS], F32, name=f"w1s{i}") for i in range(2)]
            h2a_par = [pp.tile([128, NGRP, 17], BF16, name=f"h2a{i}") for i in range(2)]
            ls2_par = [pp.tile([128, NGRP], F32, name=f"ls2{i}") for i in range(2)]
            w2s_par = [pp.tile([128, NGRP, 1], F32, name=f"w2s{i}") for i in range(2)]
            z1_par = [pp.tile([128, NGRP, 64], F32, name=f"z1a{i}") for i in range(2)]
            ald2_par = [pp.tile([128, NGRP], F32, name=f"ald2{i}") for i in range(2)]
            z2_all = pp.tile([128, NGRP, HID], F32)
            t2in = dp.tile([SLOTS, TB2W], BF16)
            ar_in = dp.tile([HID, G], F32)

            for _rep in range(repeat):
              bk = _rep % 2
              ald2_sb = ald2_par[bk]
              ald_sb = ald_par[bk]
              als_sb = als_par[bk]
              h1self = h1s_par[bk]
              w1s_sb = w1s_par[bk]
              h2all = h2a_par[bk]
              ls2_sb = ls2_par[bk]
              w2s_sb = w2s_par[bk]
              z1_all = z1_par[bk]
              # ---------------- phase A
              for g0 in range(0, NGRP, 2):
                ng = min(2, NGRP - g0)
                xg = wp.tile([IN, 2 * 128], F32, tag="xg")
                nc.sync.dma_start(xg[:, 0:ng * 128],
                                  xT[:, g0 * 128:(g0 + ng) * 128])
                for gi in range(ng):
                    g = g0 + gi
                    ps = psp.tile([128, W1E], F32, tag="psA")
                    nc.tensor.matmul(ps[:], xg[:, gi * 128:(gi + 1) * 128],
                                     wext_sb[:], start=True, stop=True)
                    nc.vector.tensor_copy(ald_sb[:, g, :], ps[:, 68:72])
                    nc.vector.tensor_copy(als_sb[:, g, :], ps[:, 64:68])
                    nc.scalar.activation(h1self[:, g, :], ps[:, 0:64], ACT.Copy)
              # batched self-loop weights for layer 1: w = exp(prelu(als+ald))
              e1s = wp.tile([128, NGRP * HEADS], F32, tag="e1s")
              nc.vector.tensor_add(
                  e1s[:].rearrange("p (g h) -> p g h", h=HEADS),
                  als_sb[:], ald_sb[:])
              nc.scalar.activation(e1s[:], e1s[:], ACT.Prelu, alpha=alpha02[:])
              nc.scalar.activation(
                  w1s_sb[:].rearrange("p g h -> p (g h)"), e1s[:], ACT.Exp)

              def edge_layer(table, TBW, nfeat, nheads, ald_t, bias_sb, zout, layer,
                             src_mm=False, selfw=None, selfh=None):
                alcol = nfeat if src_mm else 16
                L = layer
                Kmax = int(meta["Kg1"].max()) if src_mm else int(Kg.max())
                Kgl = meta["Kg1"] if src_mm else Kg
                gb1 = meta["gbase1"]
                for g in range(NGRP):
                    K = int(Kgl[g])
                    gt = wp.tile([128, Kmax * TBW], BF16, tag=f"gt{L}")
                    gtv = gt[:, 0:K * TBW].rearrange("p (k e) -> p k e", e=TBW)
                    if src_mm:
                        CB = 21
                        j0 = 0
                        while j0 < K:
                            j1 = min(j0 + CB, K)
                            xs = wp.tile([65, CB * 128], BF16, tag="xs")
                            nc.sync.dma_start(
                                xs[:, 0:(j1 - j0) * 128],
                                xsrc[:, (gb1[g] + j0) * 128:(gb1[g] + j1) * 128])
                            for j4 in range(j0, j1, 7):
                                j4e = min(j4 + 7, j1)
                                nj = j4e - j4
                                psb = psp.tile([128, 7 * W1E], F32, tag="psA")
                                for j in range(j4, j4e):
                                    nc.tensor.matmul(
                                        psb[:, (j - j4) * W1E:(j - j4 + 1) * W1E],
                                        xs[:, (j - j0) * 128:(j - j0 + 1) * 128],
                                        wextb_sb[:], start=True, stop=True)
                                nc.scalar.activation(
                                    gtv[:, j4:j4e, :],
                                    psb[:, 0:nj * W1E].rearrange(
                                        "p (j e) -> p j e", e=W1E)[:, :, 0:68],
                                    ACT.Copy)
                            j0 = j1
                    else:
                      idx_t = wp.tile([128, 8 * int(Kg.max())], I16, tag="idx")
                      nc.sync.dma_start(idx_t[:, 0:8 * K],
                                        idx[:, 8 * gbase[g]: 8 * (gbase[g] + K)])
                      for c in range(4):
                        kc = int(Kgc[g, c])
                        off = int(coloff[g, c])
                        base = VIEW_BASE2[c]
                        a = 0
                        while a < kc:
                            b = min(a + COLS_PER_CALL, kc)
                            T = (b - a) * 128
                            qn = cc_ins.setdefault("qctr", [0])
                            ins_g = nc.gpsimd.dma_gather(
                                gtv[:, off + a: off + b, :],
                                table[base: base + WS, :],
                                idx_t[:, 8 * (off + a): 8 * (off + b)],
                                T, T, TBW, queue_num=qn[0] % nqueues)
                            qn[0] += 1
                            cc_ins.setdefault(f"gathers{layer}", []).append(ins_g)
                            a = b

                    # softmax weights: Prelu(bias=ald)+Exp(accum) on scalar
                    e_t = wp.tile([128, nheads * Kmax], F32, tag=f"e{L}")
                    w_t = wp.tile([128, nheads * Kmax], BF16, tag=f"w{L}")
                    den = wp.tile([128, nheads], F32, tag=f"den{L}")
                    if nheads > 1:
                        # batch heads: DVE pre-add of ald, one Prelu + one Exp,
                        # denominators via DVE reduce (cuts ACT op count 4x)
                        ev3 = e_t[:, 0:nheads * K].rearrange(
                            "p (h k) -> p h k", k=K)
                        nc.vector.tensor_tensor(
                            out=ev3,
                            in0=gtv[:, :, alcol:alcol + nheads].rearrange(
                                "p k h -> p h k"),
                            in1=ald_t[:, g, :].rearrange(
                                "p (h o) -> p h o", o=1).to_broadcast(
                                [128, nheads, K]),
                            op=mybir.AluOpType.add)
                        nc.scalar.activation(e_t[:, 0:nheads * K],
                                             e_t[:, 0:nheads * K],
                                             ACT.Prelu, alpha=alpha02[:])
                        nc.scalar.activation(w_t[:, 0:nheads * K],
                                             e_t[:, 0:nheads * K], ACT.Exp)
                        nc.vector.reduce_sum(
                            den[:],
                            w_t[:, 0:nheads * K].rearrange(
                                "p (h k) -> p h k", k=K),
                            axis=mybir.AxisListType.X)
                    else:
                        for h in range(nheads):
                            ev = e_t[:, h * K:(h + 1) * K]
                            bias_ap = ald_t[:, g:g + 1]
                            nc.scalar.activation(ev, gtv[:, :, alcol + h],
                                                 ACT.Prelu, bias=bias_ap,
                                                 alpha=alpha02[:])
                            nc.scalar.activation(w_t[:, h * K:(h + 1) * K], ev,
                                                 ACT.Exp,
                                                 accum_out=den[:, h:h + 1])
                    # self-loop contribution to the softmax denominator
                    nc.vector.tensor_add(den[:], den[:], selfw[:, g, :])
                    dinv = wp.tile([128, nheads], F32, tag=f"dinv{L}")
                    nc.vector.reciprocal(dinv[:], den[:])

                    CH = nfeat // nheads
                    m_t = wp.tile([128, Kmax * nfeat], BF16, tag=f"m{L}")
                    mv = m_t[:, 0:K * nfeat].rearrange("p (k h c) -> p k h c",
                                                       h=nheads, c=CH)
                    wv = w_t[:, 0:nheads * K].rearrange("p (h k) -> p h k", k=K)
                    for h in range(nheads):
                        nc.vector.tensor_tensor(
                            out=mv[:, :, h, :],
                            in0=gtv[:, :, h * CH:(h + 1) * CH],
                            in1=wv[:, h:h + 1, :].rearrange(
                                "p o k -> p k o").to_broadcast([128, K, CH]),
                            op=mybir.AluOpType.mult)
                    msum = wp.tile([128, nfeat], F32, tag=f"ms{L}")
                    nc.vector.reduce_sum(
                        msum[:].rearrange("p (h c) -> p h c", c=CH),
                        mv.rearrange("p k h c -> p h c k"),
                        axis=mybir.AxisListType.X)
                    # self-loop contribution to the numerator
                    sm = wp.tile([128, nfeat], F32, tag=f"sm{L}")
                    nc.vector.tensor_tensor(
                        out=sm[:].rearrange("p (h c) -> p h c", c=CH),
                        in0=selfh[:, g, 0:nfeat].rearrange("p (h c) -> p h c", c=CH),
                        in1=selfw[:, g, :].rearrange("p (h o) -> p h o", o=1)
                        .to_broadcast([128, nheads, CH]),
                        op=mybir.AluOpType.mult)
                    nc.vector.tensor_add(msum[:], msum[:], sm[:])
                    y = wp.tile([128, nfeat], F32, tag=f"y{L}")
                    nc.vector.tensor_tensor(
                        out=y[:].rearrange("p (h c) -> p h c", c=CH),
                        in0=msum[:].rearrange("p (h c) -> p h c", c=CH),
                        in1=dinv[:].rearrange("p (h o) -> p h o", o=1).to_broadcast(
                            [128, nheads, CH]),
                        op=mybir.AluOpType.mult)
                    nc.vector.tensor_add(y[:], y[:], bias_sb[:])
                    tneg = wp.tile([128, nfeat], F32, tag=f"tn{L}")
                    nc.vector.tensor_scalar_min(tneg[:], y[:], 0.0)
                    ex = wp.tile([128, nfeat], F32, tag=f"ex{L}")
                    nc.scalar.activation(ex[:], tneg[:], ACT.Exp)
                    nc.vector.tensor_scalar_add(ex[:], ex[:], -1.0)
                    nc.vector.tensor_scalar_max(y[:], y[:], 0.0)
                    nc.vector.tensor_add(zout[:, g, :], y[:], ex[:])

              # ---------------- phase B
              edge_layer(None, 68, 64, HEADS, ald_sb, b1_sb, z1_all, 1, src_mm=True,
                         selfw=w1s_sb, selfh=h1self)

              # ---------------- phase C
              for g in range(NGRP):
                pst = psp.tile([64, 128], F32, tag="psT")
                nc.tensor.transpose(pst[:], z1_all[:, g, :], ident[:])
                z1T = wp.tile([64, 128], F32, tag="z1T")
                nc.vector.tensor_copy(z1T[:], pst[:])
                ps2 = psp.tile([128, W2E], F32, tag="ps2")
                nc.tensor.matmul(ps2[:], z1T[:], w2ext_sb[:], start=True, stop=True)
                nc.scalar.activation(h2all[:, g, :], ps2[:, 0:17], ACT.Copy)
                nc.vector.tensor_copy(ald2_sb[:, g:g + 1], ps2[:, 17:18])
                nc.vector.tensor_copy(ls2_sb[:, g:g + 1], ps2[:, 16:17])
                nc.sync.dma_start(t2in[g * 128:(g + 1) * 128, 0:17], h2all[:, g, :])
              nc.sync.dma_start(t2in[12500:12544, 16:17], negf[:, :])
              # batched self-loop weights for layer 2
              e2s = wp.tile([128, NGRP], F32, tag="e2s")
              nc.vector.tensor_add(e2s[:], ls2_sb[:], ald2_sb[:])
              nc.scalar.activation(e2s[:], e2s[:], ACT.Prelu, alpha=alpha02[:])
              nc.scalar.activation(
                  w2s_sb[:].rearrange("p g h -> p (g h)"), e2s[:], ACT.Exp)
              cc2 = nc.gpsimd.collective_compute(
                "AllGather", mybir.AluOpType.bypass,
                replica_groups=[list(range(NCORES))],
                ins=[t2in[:].opt()], outs=[table2[:, :].opt()])
              cc_ins["ag2"] = cc2.ins if hasattr(cc2, "ins") else cc2
              from concourse.bass import _add_dep_helper
              for pg in cc_ins.get("prev_g2", []):
                  _add_dep_helper(cc_ins["ag2"], pg.ins, sync=True)

              # ---------------- phase D
              edge_layer(table2, TB2W, HID, 1, ald2_sb, b2_sb, z2_all, 2,
                         selfw=w2s_sb, selfh=h2all)

              # ---------------- phase E
              pool_ps = psp1.tile([HID, G], F32)
              for g0 in range(0, NGRP, 2):
                ng = min(2, NGRP - g0)
                pind = wp.tile([128, 2 * G], BF16, tag="pind")
                nc.sync.dma_start(
                    pind[:, 0:ng * G].rearrange("p (g e) -> p g e", e=G),
                    poolind[g0:g0 + ng, :, :].rearrange("g p e -> p g e"))
                for gi in range(ng):
                    g = g0 + gi
                    z2b = wp.tile([128, HID], BF16, tag="z2b")
                    nc.vector.tensor_copy(z2b[:], z2_all[:, g, :])
                    nc.tensor.matmul(pool_ps[:], z2b[:],
                                     pind[:, gi * G:(gi + 1) * G],
                                     start=(g == 0), stop=(g == NGRP - 1))
              pool_sb = pp.tile([HID, G], F32)
              nc.vector.tensor_copy(pool_sb[:], pool_ps[:])
              nc.sync.dma_start(ar_in[:], pool_sb[:])
              cc3 = nc.gpsimd.collective_compute(
                "AllReduce", mybir.AluOpType.add,
                replica_groups=[list(range(NCORES))],
                ins=[ar_in[:].opt()], outs=[ar_out[:, :].opt()])
              cc_ins["ar"] = cc3.ins if hasattr(cc3, "ins") else cc3

              pooled = pp.tile([HID, G], F32)
              dma_pool = nc.sync.dma_start(pooled[:], ar_out[:, :])
              nc.vector.tensor_mul(pooled[:], pooled[:], invc_sb[:])
              ps_out = psp1.tile([1, G], F32)
              nc.tensor.matmul(ps_out[:], wc_sb[:], pooled[:], start=True, stop=True)
              res = pp.tile([1, G], F32)
              nc.vector.tensor_tensor(out=res[:], in0=ps_out[:],
                                    in1=bc_sb[:].to_broadcast([1, G]),
                                    op=mybir.AluOpType.add)
              nc.sync.dma_start(out[:, :], res[:])

              from concourse.bass import _add_dep_helper
              for gg in cc_ins.get("gathers2", []):
                  _add_dep_helper(gg.ins, cc_ins["ag2"], sync=True)
              _add_dep_helper(dma_pool.ins, cc_ins["ar"], sync=True)
              cc_ins["prev_g2"] = cc_ins.get("gathers2", [])[-nqueues:]
              cc_ins.pop("gathers2", None)
              cc_ins.pop("ag2", None)
              cc_ins.pop("ar", None)

    nc.finalize()
    return nc


# ---------------------------------------------------------------- runner
def _make_spmd_fn(nc, n_cores=8):
    import jax
    from concourse.bass2jax import (_bass_exec_p, install_neuronx_cc_hook,
                                    partition_id_tensor)
    from jax.sharding import Mesh, PartitionSpec, NamedSharding
    from jax.experimental.shard_map import shard_map

    install_neuronx_cc_hook()
    partition_name = nc.partition_id_tensor.name if nc.partition_id_tensor else None
    in_names, out_names, out_avals, zero_outs = [], [], [], []
    for alloc in nc.m.functions[0].allocations:
        if not isinstance(alloc, mybir.MemoryLocationSet):
            continue
        name = alloc.memorylocations[0].name
        if alloc.kind == "ExternalInput":
            if name != partition_name:
                in_names.append(name)
        elif alloc.kind == "ExternalOutput":
            out_names.append(name)
            shape = tuple(alloc.tensor_shape)
            dtype = mybir.dt.np(alloc.dtype)
            out_avals.append(jax.core.ShapedArray(shape, dtype))
            zero_outs.append(np.zeros(shape, dtype))
    n_params = len(in_names)
    all_in = list(in_names) + list(out_names)
    if partition_name is not None:
        all_in.append(partition_name)

    def _body(*args):
        operands = list(args)
        if partition_name is not None:
            operands.append(partition_id_tensor())
        return tuple(_bass_exec_p.bind(
            *operands, out_avals=tuple(out_avals), in_names=tuple(all_in),
            out_names=tuple(out_names), lowering_input_output_aliases=(),
            sim_require_finite=False, sim_require_nnan=False, nc=nc))

    devices = jax.devices()[:n_cores]
    mesh = Mesh(np.asarray(devices), ("core",))
    specs = (PartitionSpec("core"),)
    sharded = jax.jit(
        shard_map(_body, mesh=mesh, in_specs=specs * (n_params + len(out_names)),
                  out_specs=specs * len(out_names), check_rep=False),
        keep_unused=True)

    def place(in_maps):
        sh = NamedSharding(mesh, PartitionSpec("core"))
        placed = [jax.device_put(
            np.concatenate([np.asarray(in_maps[c][nm]) for c in range(n_cores)], axis=0), sh)
            for nm in in_names]
        jax.block_until_ready(placed)
        return placed

    def fn(in_maps_or_placed):
        if isinstance(in_maps_or_placed, list) and in_maps_or_placed and                 isinstance(in_maps_or_placed[0], dict):
            concat = place(in_maps_or_placed)
        else:
            concat = in_maps_or_placed
        zeros = [np.zeros((n_cores * z.shape[0], *z.shape[1:]), z.dtype)
                 for z in zero_outs]
        outs = sharded(*concat, *zeros)
        jax.block_until_ready(outs)
        return [{nm: np.asarray(outs[i]).reshape(n_cores, *out_avals[i].shape)[c]
                 for i, nm in enumerate(out_names)} for c in range(n_cores)]

    fn.place = place
    return fn



def make_inputs(inputs):
    x = np.asarray(inputs["x"], np.float32)
    edge_index = np.asarray(inputs["edge_index"], np.int64)
    batch = np.asarray(inputs["batch"], np.int64)
    W1 = np.asarray(inputs["W1"], np.float32)
    a1_src = np.asarray(inputs["a1_src"], np.float32)
    a1_dst = np.asarray(inputs["a1_dst"], np.float32)
    W2 = np.asarray(inputs["W2"], np.float32)
    a2_src = np.asarray(inputs["a2_src"], np.float32)
    a2_dst = np.asarray(inputs["a2_dst"], np.float32)

    meta, idx16, xT, poolind, invcnt, xsrc = _host_prep2(x, edge_index, batch)

    wext = np.zeros((IN, 72), np.float32)
    wext[:, 0:64] = W1
    for h in range(HEADS):
        wext[:, 64 + h] = W1[:, h * HID:(h + 1) * HID] @ a1_src[h]
        wext[:, 68 + h] = W1[:, h * HID:(h + 1) * HID] @ a1_dst[h]
    w2ext = np.zeros((IN, 18), np.float32)
    w2ext[:, 0:16] = W2
    w2ext[:, 16] = W2 @ a2_src[0]
    w2ext[:, 17] = W2 @ a2_dst[0]
    wextb = np.zeros((65, 72), np.float32)
    wextb[0:64, :] = wext
    wextb[64, 64:68] = -1e30
    wextb = wextb.astype(ml_dtypes.bfloat16)
    b1t = np.tile(np.asarray(inputs["b1"], np.float32)[None, :], (128, 1))
    b2t = np.tile(np.asarray(inputs["b2"], np.float32)[None, :], (128, 1))

    in_maps = []
    for c in range(NCORES):
        in_maps.append(dict(
            xT=xT[c], wext=wext, w2ext=w2ext, b1t=b1t, b2t=b2t,
            idx=idx16[c], poolind=poolind[c], invcnt=invcnt,
            wc=np.asarray(inputs["Wc"], np.float32).reshape(HID, 1),
            bc=np.asarray(inputs["bc"], np.float32).reshape(1, 1),
            xsrc=xsrc[c], wextb=wextb,
            negf=np.full((44, 1), -1e30, ml_dtypes.bfloat16)))
    return meta, in_maps


def kernel(**inputs):
    meta, in_maps = make_inputs(inputs)
    nc = _build(meta)
    fn = _make_spmd_fn(nc)
    placed = fn.place(in_maps)
    prev = None
    outv = None
    for _ in range(3):
        res = fn(placed)
        cur = res[0]["out"].reshape(G, 1).astype(np.float32)
        if prev is not None and np.allclose(cur, prev, rtol=1e-5, atol=1e-8):
            outv = cur
            break
        prev = cur
    if outv is None:
        outv = prev
    kernel._last_fn = fn
    kernel._last_meta = meta
    kernel._last_in_maps = in_maps
    return outv

